# revision 10
# baseline (speedup 1.0000x reference)
"""Single-head attention (B=8, S=2048, H=768, D=64) on 8 TRN2 NeuronCores.

Data-parallel over batch: core b computes batch element b end to end, no
collectives. Host pre-transposes inputs to [H, S] bf16 (and pre-arranges
the weights into the SBUF layout) so every DMA is contiguous and every
matmul contraction is on the partition axis.

The Tile framework list-schedules each engine by (readiness, emission
priority), so emission order here is a priority hint.  It is arranged
around three measured facts:
  * ACT is the softmax floor (32 exps of [128,1024] at ~1.1us each, plus
    ~0.2us of semaphore latency per exp); it must start ASAP and never
    starve.  Everything the exp stream depends on (projections feeding
    scores) is emitted at the earliest priority its DMA can satisfy.
  * back-to-back matmuls that share lhsT skip the ~120ns LDWEIGHTS
    exposure, so weight-sharing matmuls are emitted adjacently
    (projection chunk pairs h-major, score sub-pairs, AV pairs).
  * input DMA sustains ~280 GB/s aggregate; DMAs are emitted in
    consumption-deadline order (w, q c0/c1, k c0-c3, q c2/c3, v).

Per-core dataflow (matmuls bf16 x bf16 -> f32 PSUM):
  qT[64,S], kT[64,S], vT[64,S] = W^T X    (6 h-tile accumulating matmuls
                                           per 512-col chunk)
  ps[128,1024] = kT_tile^T qT_half        (scores^T, sk on partitions)
  pth = exp(ps/8 + mask_bias)             (ACT, bf16, mask bias fused)
  vE[128, 65] tiles: PE-transpose of vT tiles; col 64 preset to ones
  po[c][65,512] += vE_t^T pth_t           (row 64 = softmax denominator)
  oT[65, S] f32 DMA'd out raw; host computes (oT[:64]/oT[64]).T

PSUM (8 banks): ps [128,1024] x2 (4 banks) + o0-o3 (4 banks) with tag
rotation  o0: Q0,Q2,V0,px0-3,po0   o1: K0,K2,V1,px4-7,po1
          o2: Q1,Q3,V2,px8-11,po2  o3: K1,K3,V3,px12-15,po3
"""

import os
from contextlib import ExitStack

import numpy as np
import ml_dtypes

import concourse.bass as bass
import concourse.mybir as mybir
import concourse.tile as tile
from concourse import bacc
from concourse.bass_utils import run_bass_kernel_spmd
from concourse.masks import make_identity

S, H, D = 2048, 768, 64
P = 128
NT = S // P      # 16 sk tiles
HT = H // P      # 6 h tiles
CH = 512         # matmul moving-dim chunk
NCH = S // CH    # 4
HALF = 1024      # sq half width (exp tile width)
BF = mybir.dt.bfloat16
F32 = mybir.dt.float32
AF = mybir.ActivationFunctionType

LAST_RESULT = None  # BassKernelResults of the most recent run (for test.py)


def _build(debug=False):
    nc = bacc.Bacc()
    qT_d = nc.declare_dram_parameter("qT", [H, S], BF, isOutput=False)
    kT_d = nc.declare_dram_parameter("kT", [H, S], BF, isOutput=False)
    vT_d = nc.declare_dram_parameter("vT", [H, S], BF, isOutput=False)
    # weights pre-arranged on host to the SBUF layout: [p, h_tile, n]
    wqk_d = nc.declare_dram_parameter("wqk", [P, HT * P], BF, isOutput=False)
    wv_d = nc.declare_dram_parameter("wv", [P, HT * D], BF, isOutput=False)
    bq_d = nc.declare_dram_parameter("bq", [D, 1], F32, isOutput=False)
    bk_d = nc.declare_dram_parameter("bk", [D, 1], F32, isOutput=False)
    bv_d = nc.declare_dram_parameter("bv", [D, 1], F32, isOutput=False)
    mb_d = nc.declare_dram_parameter("mb", [P, NT], F32, isOutput=False)
    o_d = nc.declare_dram_parameter("o", [D + 1, S], F32, isOutput=True)

    with ExitStack() as ctx:
        tc = ctx.enter_context(tile.TileContext(nc))
        consts = ctx.enter_context(tc.tile_pool(name="consts", bufs=1))
        stage = ctx.enter_context(tc.tile_pool(name="stage", bufs=1))
        persist = ctx.enter_context(tc.tile_pool(name="persist", bufs=1))
        ppool = ctx.enter_context(tc.tile_pool(name="ppool", bufs=28))
        ostage = ctx.enter_context(tc.tile_pool(name="ostage", bufs=4))
        psA = ctx.enter_context(tc.tile_pool(name="psA", bufs=2, space="PSUM"))
        psO = ctx.enter_context(tc.tile_pool(name="psO", bufs=1, space="PSUM"))

        # ---- t=0: preload the ACT exp table with a dummy exp so the
        # ~1.4us table load happens during the DMA head, not before exp #1
        scr = consts.tile([P, 1], F32, tag="scr")
        nc.gpsimd.memset(scr, 0.0)
        dum = consts.tile([P, 1], BF, tag="dum")
        nc.scalar.activation(out=dum, in_=scr, func=AF.Exp, scale=1.0)

        # ---- consts + weights first on the sync ring (all contiguous)
        mb_sb = consts.tile([P, NT], F32, tag="mb")
        nc.sync.dma_start(out=mb_sb, in_=mb_d[:, :])
        bq_sb = consts.tile([D, 1], F32, tag="bq")
        nc.sync.dma_start(out=bq_sb, in_=bq_d[:, :])
        bk_sb = consts.tile([D, 1], F32, tag="bk")
        nc.sync.dma_start(out=bk_sb, in_=bk_d[:, :])
        bv_sb = consts.tile([D, 1], F32, tag="bv")
        nc.sync.dma_start(out=bv_sb, in_=bv_d[:, :])
        w_sb = consts.tile([P, HT, P], BF, tag="w")  # [Wq|Wk] h-tiles
        nc.sync.dma_start(out=w_sb[:, :, :], in_=wqk_d[:, :])
        wv_sb = consts.tile([P, HT, D], BF, tag="wv")
        nc.sync.dma_start(out=wv_sb[:, :, :], in_=wv_d[:, :])
        ident_bf = consts.tile([P, P], BF, tag="ident_bf")
        make_identity(nc, ident_bf)

        # ---- input staging: [128,512] chunk pieces, emitted (= DMA'd)
        # in consumption-deadline order.
        st = {}

        def stage_in(t, h, c):
            tl = stage.tile(
                [P, CH],
                BF,
                tag="in",
                bufs=72,
                name=f"st_{t}{h}{c}",
            )
            nc.sync.dma_start(
                out=tl,
                in_={"q": qT_d, "k": kT_d, "v": vT_d}[t][
                    h * P : (h + 1) * P, c * CH : (c + 1) * CH
                ],
            )
            st[t, h, c] = tl

        for h in range(HT):          # q chunks 0-1, h-interleaved
            stage_in("q", h, 0)
            stage_in("q", h, 1)
        for c in range(4):           # all of k
            for h in range(HT):
                stage_in("k", h, c)
        for c in range(2, 4):        # q chunks 2-3 (half-1 scores rhs)
            for h in range(HT):
                stage_in("q", h, c)
        for c in range(4):           # v last
            for h in range(HT):
                stage_in("v", h, c)

        # ---- persistent SBUF ----
        qT_sb = persist.tile([D, S], BF, tag="qT")
        kT_sb = persist.tile([D, S], BF, tag="kT")
        vTp_sb = persist.tile([D, S], BF, tag="vTp")
        vE_sb = persist.tile([P, NT * (D + 1)], BF, tag="vE")
        nc.gpsimd.memset(vE_sb, 1.0)  # ones col (col 64 of each 65-tile)

        # ---- helper blocks -------------------------------------------
        W = {
            "q": (w_sb, slice(0, D), bq_sb, qT_sb),
            "k": (w_sb, slice(D, P), bk_sb, kT_sb),
            "v": (wv_sb, slice(0, D), bv_sb, vTp_sb),
        }

        def proj_pair(t, c0, tagA, tagB):
            """Chunks c0, c0+1 of projection t, h-major interleaved so
            consecutive matmuls share lhsT (skips LDWEIGHTS)."""
            w, wcols, bias, dst = W[t]
            ppa = psO.tile([D, CH], F32, tag=tagA, name=f"pp_{t}{c0}")
            ppb = psO.tile([D, CH], F32, tag=tagB, name=f"pp_{t}{c0 + 1}")
            for h in range(HT):
                for pp, c in ((ppa, c0), (ppb, c0 + 1)):
                    nc.tensor.matmul(
                        pp,
                        lhsT=w[:, h, wcols],
                        rhs=st[t, h, c],
                        start=(h == 0),
                        stop=(h == HT - 1),
                    )
            for pp, c in ((ppa, c0), (ppb, c0 + 1)):
                nc.vector.tensor_scalar_add(
                    out=dst[:, c * CH : (c + 1) * CH], in0=pp, scalar1=bias
                )

        pth = {}

        def scores_exp(t, half):
            ps = psA.tile([P, HALF], F32, tag="ps", name=f"ps{t}_{half}")
            for sub in range(2):
                nc.tensor.matmul(
                    ps[:, sub * CH : (sub + 1) * CH],
                    lhsT=kT_sb[:, t * P : (t + 1) * P],
                    rhs=qT_sb[
                        :, half * HALF + sub * CH : half * HALF + (sub + 1) * CH
                    ],
                    start=True,
                    stop=True,
                )
            pt = ppool.tile([P, HALF], BF, tag="pT", name=f"pt{t}_{half}")
            nc.scalar.activation(
                out=pt, in_=ps, func=AF.Exp, bias=mb_sb[:, t : t + 1], scale=0.125
            )
            pth[t, half] = pt

        def v_xpose(t, tag):
            """vT tile [64,128] -> vE tile [128,64] via PE transpose; the
            PSUM staging tile borrows an o-bank rotation slot right after
            the V-projection chunk that produced its input."""
            px = psO.tile([P, D], BF, tag=tag, name=f"px{t}")
            nc.tensor.transpose(
                px,
                in_=vTp_sb[:, t * P : (t + 1) * P],
                identity=ident_bf[:D, :D],
            )
            nc.vector.tensor_copy(
                out=vE_sb[:, t * (D + 1) : t * (D + 1) + D], in_=px
            )

        po = {}

        def mk_po(c):
            po[c] = psO.tile([D + 1, CH], F32, tag=f"o{c}", name=f"po{c}")

        def av(t, c):
            nc.tensor.matmul(
                po[c],
                lhsT=vE_sb[:, t * (D + 1) : (t + 1) * (D + 1)],
                rhs=pth[t, c // 2][:, (c % 2) * CH : (c % 2 + 1) * CH],
                start=(t == 0),
                stop=(t == NT - 1),
            )

        def emit_out(c):
            ot = ostage.tile([D + 1, CH], F32, tag="ot", name=f"ot{c}")
            nc.vector.tensor_copy(out=ot, in_=po[c])
            nc.sync.dma_start(out=o_d[:, c * CH : (c + 1) * CH], in_=ot)

        # ---- schedule (priority order) --------------------------------
        proj_pair("q", 0, "o0", "o2")
        proj_pair("k", 0, "o1", "o3")
        scores_exp(0, 0)
        scores_exp(1, 0)
        scores_exp(2, 0)
        scores_exp(3, 0)
        proj_pair("k", 2, "o1", "o3")
        scores_exp(4, 0)
        scores_exp(5, 0)
        scores_exp(6, 0)
        scores_exp(7, 0)
        proj_pair("q", 2, "o0", "o2")
        for t in range(8, NT):
            scores_exp(t, 0)
        proj_pair("v", 0, "o0", "o1")
        for t in range(4):
            v_xpose(t, "o0")
        for t in range(4, 8):
            v_xpose(t, "o1")
        mk_po(0)
        mk_po(1)
        # half-1 boundary: scores first, then the t=0/1 AVs of half 0
        scores_exp(0, 1)
        scores_exp(1, 1)
        av(0, 0)
        av(0, 1)
        av(1, 0)
        av(1, 1)
        proj_pair("v", 2, "o2", "o3")
        for t in range(8, 12):
            v_xpose(t, "o2")
        for t in range(12, 16):
            v_xpose(t, "o3")
        mk_po(2)
        mk_po(3)
        # steady phase: each exp-cycle carries its own AVs plus the
        # two-cycles-back half-1 AVs
        for j in range(2, NT):
            scores_exp(j, 1)
            av(j, 0)
            av(j, 1)
            av(j - 2, 2)
            av(j - 2, 3)
        emit_out(0)
        emit_out(1)
        for t in (NT - 2, NT - 1):
            av(t, 2)
            av(t, 3)
        emit_out(2)
        emit_out(3)

        if debug:
            for nm, tl in [
                ("dbg_qT", qT_sb),
                ("dbg_kT", kT_sb),
                ("dbg_vTp", vTp_sb),
                ("dbg_vE", vE_sb),
            ]:
                dd = nc.declare_dram_parameter(
                    nm, list(tl.shape), BF, isOutput=True
                )
                nc.sync.dma_start(out=dd[:, :], in_=tl)

    return nc


_NC = None


def kernel(query, key, value, mask, Wq, bq, Wk, bk, Wv, bv):
    global _NC, LAST_RESULT
    bf16 = ml_dtypes.bfloat16
    B = query.shape[0]
    assert B == 8

    if _NC is None:
        _NC = _build(debug=bool(os.environ.get("KERNEL_DEBUG")))
        _NC.finalize()

    # weights pre-arranged to the SBUF tile layout [p, h_tile, n]
    wqk = np.concatenate([np.asarray(Wq), np.asarray(Wk)], axis=1)  # [H,128]
    wqk = np.ascontiguousarray(
        wqk.reshape(HT, P, P).transpose(1, 0, 2).reshape(P, HT * P).astype(bf16)
    )
    wv = np.ascontiguousarray(
        np.asarray(Wv)
        .reshape(HT, P, D)
        .transpose(1, 0, 2)
        .reshape(P, HT * D)
        .astype(bf16)
    )
    bq_h = np.asarray(bq, np.float32).reshape(D, 1)
    bk_h = np.asarray(bk, np.float32).reshape(D, 1)
    bv_h = np.asarray(bv, np.float32).reshape(D, 1)

    in_maps = []
    for b in range(B):
        mb = ((np.asarray(mask[b], np.float32) - 1.0) * 1e9).reshape(NT, P).T
        in_maps.append(
            {
                "qT": np.ascontiguousarray(np.asarray(query[b]).T.astype(bf16)),
                "kT": np.ascontiguousarray(np.asarray(key[b]).T.astype(bf16)),
                "vT": np.ascontiguousarray(np.asarray(value[b]).T.astype(bf16)),
                "wqk": wqk,
                "wv": wv,
                "bq": bq_h,
                "bk": bk_h,
                "bv": bv_h,
                "mb": np.ascontiguousarray(mb),
            }
        )

    res = run_bass_kernel_spmd(
        _NC,
        in_maps,
        core_ids=list(range(8)),
        trace=bool(os.environ.get("KERNEL_TRACE")),
    )
    LAST_RESULT = res
    out = np.empty((B, S, D), np.float32)
    for b in range(B):
        oT = np.asarray(res.results[b]["o"])  # [65, S] f32, unnormalized
        out[b] = (oT[:D] / oT[D : D + 1]).T
    return out


# revision 22
# speedup vs baseline: 1.2600x; 1.2600x over previous
"""Single-head attention (B=8, S=2048, H=768, D=64) on 8 TRN2 NeuronCores.

Data-parallel over batch: core b computes batch element b end to end, no
collectives. Host pre-transposes inputs to [H, S] bf16 (and pre-arranges
the weights into the SBUF layout) so every DMA is contiguous and every
matmul contraction is on the partition axis.

The Tile framework list-schedules each engine by (readiness, emission
priority), so emission order here is a priority hint.  It is arranged
around three measured facts:
  * ACT is the softmax floor (32 exps of [128,1024] at ~1.1us each, plus
    ~0.2us of semaphore latency per exp); it must start ASAP and never
    starve.  Everything the exp stream depends on (projections feeding
    scores) is emitted at the earliest priority its DMA can satisfy.
  * back-to-back matmuls that share lhsT skip the ~120ns LDWEIGHTS
    exposure, so weight-sharing matmuls are emitted adjacently
    (projection chunk pairs h-major, score sub-pairs, AV pairs).
  * input DMA sustains ~280 GB/s aggregate; DMAs are emitted in
    consumption-deadline order (w, q c0/c1, k c0-c3, q c2/c3, v).

Per-core dataflow (matmuls bf16 x bf16 -> f32 PSUM):
  qT[64,S], kT[64,S], vT[64,S] = W^T X    (6 h-tile accumulating matmuls
                                           per 512-col chunk)
  ps[128,1024] = kT_tile^T qT_half        (scores^T, sk on partitions)
  pth = exp(ps/8 + mask_bias)             (ACT, bf16, mask bias fused)
  vE[128, 65] tiles: PE-transpose of vT tiles; col 64 preset to ones
  po[c][65,512] += vE_t^T pth_t           (row 64 = softmax denominator)
  oT[65, S] f32 DMA'd out raw; host computes (oT[:64]/oT[64]).T

PSUM (8 banks): ps [128,1024] x2 (4 banks) + o0-o3 (4 banks) with tag
rotation  o0: Q0,Q2,V0,px0-3,po0   o1: K0,K2,V1,px4-7,po1
          o2: Q1,Q3,V2,px8-11,po2  o3: K1,K3,V3,px12-15,po3
"""

import os
from contextlib import ExitStack

import numpy as np
import ml_dtypes

import concourse.bass as bass
import concourse.mybir as mybir
import concourse.tile as tile
from concourse import bacc
from concourse.bass_utils import run_bass_kernel_spmd
from concourse.masks import make_identity

S, H, D = 2048, 768, 64
P = 128
NT = S // P      # 16 sk tiles
HT = H // P      # 6 h tiles
CH = 512         # matmul moving-dim chunk
NCH = S // CH    # 4
HALF = 1024      # sq half width (exp tile width)
BF = mybir.dt.bfloat16
F32 = mybir.dt.float32
AF = mybir.ActivationFunctionType

LAST_RESULT = None  # BassKernelResults of the most recent run (for test.py)


def _build(debug=False):
    nc = bacc.Bacc()
    qT_d = nc.declare_dram_parameter("qT", [H, S], BF, isOutput=False)
    kT_d = nc.declare_dram_parameter("kT", [H, S], BF, isOutput=False)
    vT_d = nc.declare_dram_parameter("vT", [H, S], BF, isOutput=False)
    # weights pre-arranged on host to the SBUF layout: [p, h_tile, n]
    wqk_d = nc.declare_dram_parameter("wqk", [P, HT * P], BF, isOutput=False)
    wv_d = nc.declare_dram_parameter("wv", [P, HT * D], BF, isOutput=False)
    bq_d = nc.declare_dram_parameter("bq", [D, 1], F32, isOutput=False)
    bk_d = nc.declare_dram_parameter("bk", [D, 1], F32, isOutput=False)
    bv_d = nc.declare_dram_parameter("bv", [D, 1], F32, isOutput=False)
    mb_d = nc.declare_dram_parameter("mb", [P, NT], F32, isOutput=False)
    o_d = nc.declare_dram_parameter("o", [D + 1, S], F32, isOutput=True)

    with ExitStack() as ctx:
        tc = ctx.enter_context(tile.TileContext(nc))
        consts = ctx.enter_context(tc.tile_pool(name="consts", bufs=1))
        stage = ctx.enter_context(tc.tile_pool(name="stage", bufs=1))
        persist = ctx.enter_context(tc.tile_pool(name="persist", bufs=1))
        ppool = ctx.enter_context(tc.tile_pool(name="ppool", bufs=32))
        ostage = ctx.enter_context(tc.tile_pool(name="ostage", bufs=4))
        psA = ctx.enter_context(tc.tile_pool(name="psA", bufs=2, space="PSUM"))
        psO = ctx.enter_context(tc.tile_pool(name="psO", bufs=1, space="PSUM"))

        # ---- t=0: preload the ACT exp table with a dummy exp so the
        # ~1.4us table load happens during the DMA head, not before exp #1
        scr = consts.tile([P, 1], F32, tag="scr")
        nc.gpsimd.memset(scr, 0.0)
        dum = consts.tile([P, 1], BF, tag="dum")
        nc.scalar.activation(out=dum, in_=scr, func=AF.Exp, scale=1.0)

        # ---- PE clock warm-up.  The TRN2 tensor engine boosts to its
        # full 2.4 GHz only after a ~3us gap-free execution stretch, and
        # the boost lands ~10us after that stretch ("arming"); a DMA- or
        # exp-paced stream never supplies it, leaving the whole kernel at
        # 1.2 GHz (measured: same build, 86us vs 111us).  This chain of
        # back-to-back matmuls on a junk tile (same lhsT -> LDWEIGHTS
        # skipped; accumulation chain -> zero stalls) arms the boost as
        # early as the engines come up, while the PE is idle anyway.
        wj = consts.tile([P, P], BF, tag="wj")
        nc.gpsimd.memset(wj, 0.0)
        wrm = psA.tile([P, P], F32, tag="ps", name="wrm")
        for i in range(36):
            nc.tensor.matmul(
                wrm,
                lhsT=wj,
                rhs=wj,
                start=(i == 0),
                stop=(i == 35),
            )

        # ---- consts + weights first on the sync ring (all contiguous)
        mb_sb = consts.tile([P, NT], F32, tag="mb")
        nc.sync.dma_start(out=mb_sb, in_=mb_d[:, :])
        bq_sb = consts.tile([D, 1], F32, tag="bq")
        nc.sync.dma_start(out=bq_sb, in_=bq_d[:, :])
        bk_sb = consts.tile([D, 1], F32, tag="bk")
        nc.sync.dma_start(out=bk_sb, in_=bk_d[:, :])
        bv_sb = consts.tile([D, 1], F32, tag="bv")
        nc.sync.dma_start(out=bv_sb, in_=bv_d[:, :])
        w_sb = consts.tile([P, HT, P], BF, tag="w")  # [Wq|Wk] h-tiles
        nc.sync.dma_start(out=w_sb[:, :, :], in_=wqk_d[:, :])
        wv_sb = consts.tile([P, HT, D], BF, tag="wv")
        nc.sync.dma_start(out=wv_sb[:, :, :], in_=wv_d[:, :])
        ident_bf = consts.tile([P, P], BF, tag="ident_bf")
        make_identity(nc, ident_bf)



        # ---- input staging: [128,512] chunk pieces, emitted (= DMA'd)
        # in consumption-deadline order.
        st = {}

        def stage_in(t, h, c):
            tl = stage.tile(
                [P, CH],
                BF,
                tag="in",
                bufs=72,
                name=f"st_{t}{h}{c}",
            )
            nc.sync.dma_start(
                out=tl,
                in_={"q": qT_d, "k": kT_d, "v": vT_d}[t][
                    h * P : (h + 1) * P, c * CH : (c + 1) * CH
                ],
            )
            st[t, h, c] = tl

        for h in range(HT):          # q c0 + k c0 together: first scores
            stage_in("q", h, 0)
            stage_in("k", h, 0)
        for h in range(HT):
            stage_in("q", h, 1)
        for c in range(1, 4):        # rest of k (kT tiles for half-0 scores)
            for h in range(HT):
                stage_in("k", h, c)
        for c in range(2, 4):        # q chunks 2-3 (half-1 scores rhs)
            for h in range(HT):
                stage_in("q", h, c)
        for c in range(4):           # v last
            for h in range(HT):
                stage_in("v", h, c)

        # ---- persistent SBUF ----
        qT_sb = persist.tile([D, S], BF, tag="qT")
        kT_sb = persist.tile([D, S], BF, tag="kT")
        vTp_sb = persist.tile([D, S], BF, tag="vTp")
        vE_sb = persist.tile([P, NT * (D + 1)], BF, tag="vE")
        nc.gpsimd.memset(vE_sb, 1.0)  # ones col (col 64 of each 65-tile)

        # ---- helper blocks -------------------------------------------
        W = {
            "q": (w_sb, slice(0, D), bq_sb, qT_sb),
            "k": (w_sb, slice(D, P), bk_sb, kT_sb),
            "v": (wv_sb, slice(0, D), bv_sb, vTp_sb),
        }

        def proj(t, c, tag):
            """One 512-col projection chunk (6 accumulating matmuls +
            bias add)."""
            w, wcols, bias, dst = W[t]
            pp = psO.tile([D, CH], F32, tag=tag, name=f"pp_{t}{c}")
            for h in range(HT):
                nc.tensor.matmul(
                    pp,
                    lhsT=w[:, h, wcols],
                    rhs=st[t, h, c],
                    start=(h == 0),
                    stop=(h == HT - 1),
                )
            nc.vector.tensor_scalar_add(
                out=dst[:, c * CH : (c + 1) * CH], in0=pp, scalar1=bias
            )

        def proj_pair(t, c0, tagA, tagB):
            """Chunks c0, c0+1 of projection t, h-major interleaved so
            consecutive matmuls share lhsT (skips LDWEIGHTS)."""
            w, wcols, bias, dst = W[t]
            ppa = psO.tile([D, CH], F32, tag=tagA, name=f"pp_{t}{c0}")
            ppb = psO.tile([D, CH], F32, tag=tagB, name=f"pp_{t}{c0 + 1}")
            for h in range(HT):
                for pp, c in ((ppa, c0), (ppb, c0 + 1)):
                    nc.tensor.matmul(
                        pp,
                        lhsT=w[:, h, wcols],
                        rhs=st[t, h, c],
                        start=(h == 0),
                        stop=(h == HT - 1),
                    )
            for pp, c in ((ppa, c0), (ppb, c0 + 1)):
                nc.vector.tensor_scalar_add(
                    out=dst[:, c * CH : (c + 1) * CH], in0=pp, scalar1=bias
                )

        pth = {}

        def scores_exp(t, half, split=False):
            """One [128,1024] scores+exp tile.  split=True runs the exp in
            two 512-wide pieces so ACT can start on sub 0 while sub 1's
            matmul (needing the next qT chunk) is still pending — used for
            the first tiles to pull the exp-stream start earlier."""
            ps = psA.tile([P, HALF], F32, tag="ps", name=f"ps{t}_{half}")
            pt = ppool.tile([P, HALF], BF, tag="pT", name=f"pt{t}_{half}")
            for sub in range(2):
                cs = slice(sub * CH, (sub + 1) * CH)
                nc.tensor.matmul(
                    ps[:, cs],
                    lhsT=kT_sb[:, t * P : (t + 1) * P],
                    rhs=qT_sb[
                        :, half * HALF + sub * CH : half * HALF + (sub + 1) * CH
                    ],
                    start=True,
                    stop=True,
                )
                if split:
                    nc.scalar.activation(
                        out=pt[:, cs],
                        in_=ps[:, cs],
                        func=AF.Exp,
                        bias=mb_sb[:, t : t + 1],
                        scale=0.125,
                    )
            if not split:
                nc.scalar.activation(
                    out=pt,
                    in_=ps,
                    func=AF.Exp,
                    bias=mb_sb[:, t : t + 1],
                    scale=0.125,
                )
            pth[t, half] = pt

        def v_xpose(t, tag):
            """vT tile [64,128] -> vE tile [128,64] via PE transpose; the
            PSUM staging tile borrows an o-bank rotation slot right after
            the V-projection chunk that produced its input."""
            px = psO.tile([P, D], BF, tag=tag, name=f"px{t}")
            nc.tensor.transpose(
                px,
                in_=vTp_sb[:, t * P : (t + 1) * P],
                identity=ident_bf[:D, :D],
            )
            nc.vector.tensor_copy(
                out=vE_sb[:, t * (D + 1) : t * (D + 1) + D], in_=px
            )

        po = {}

        def mk_po(c):
            po[c] = psO.tile([D + 1, CH], F32, tag=f"o{c}", name=f"po{c}")

        def av(t, c):
            nc.tensor.matmul(
                po[c],
                lhsT=vE_sb[:, t * (D + 1) : (t + 1) * (D + 1)],
                rhs=pth[t, c // 2][:, (c % 2) * CH : (c % 2 + 1) * CH],
                start=(t == 0),
                stop=(t == NT - 1),
            )

        def emit_out(c):
            ot = ostage.tile([D + 1, CH], F32, tag="ot", name=f"ot{c}")
            nc.vector.tensor_copy(out=ot, in_=po[c])
            nc.sync.dma_start(out=o_d[:, c * CH : (c + 1) * CH], in_=ot)

        # ---- schedule (priority order) --------------------------------
        # Head: the first score tile's inputs (Q0, K0) get top priority,
        # split exps pull the ACT stream start as early as the DMA allows;
        # steady phase: each exp-cycle carries a bounded amount of AV work
        # (so the PE idles briefly right when a ps slot frees and responds
        # to the next scores matmul immediately); V/px in the natural
        # exp-stream slack; leftovers at the tail.
        proj("q", 0, "o0")
        proj("k", 0, "o1")
        proj("q", 1, "o2")
        scores_exp(0, 0)
        proj("k", 1, "o3")
        scores_exp(1, 0)
        scores_exp(2, 0)
        scores_exp(3, 0)
        proj_pair("k", 2, "o1", "o3")
        for t in range(4, 8):
            scores_exp(t, 0)
        proj_pair("q", 2, "o0", "o2")
        for t in range(8, NT):
            scores_exp(t, 0)
        proj_pair("v", 0, "o0", "o1")
        for t in range(4):
            v_xpose(t, "o0")
        for t in range(4, 8):
            v_xpose(t, "o1")
        mk_po(0)
        mk_po(1)
        scores_exp(0, 1)
        scores_exp(1, 1)
        av(0, 0)
        av(0, 1)
        av(1, 0)
        av(1, 1)
        proj_pair("v", 2, "o2", "o3")
        for t in range(8, 12):
            v_xpose(t, "o2")
        for t in range(12, 16):
            v_xpose(t, "o3")
        mk_po(2)
        mk_po(3)
        for j in range(2, NT):
            scores_exp(j, 1)
            av(j, 0)
            av(j, 1)
            av(j - 2, 2)
            av(j - 2, 3)
        emit_out(0)
        emit_out(1)
        for t in (NT - 2, NT - 1):
            av(t, 2)
            av(t, 3)
        emit_out(2)
        emit_out(3)

        if debug:
            for nm, tl in [
                ("dbg_qT", qT_sb),
                ("dbg_kT", kT_sb),
                ("dbg_vTp", vTp_sb),
                ("dbg_vE", vE_sb),
            ]:
                dd = nc.declare_dram_parameter(
                    nm, list(tl.shape), BF, isOutput=True
                )
                nc.sync.dma_start(out=dd[:, :], in_=tl)

    return nc


_NC = None


def kernel(query, key, value, mask, Wq, bq, Wk, bk, Wv, bv):
    global _NC, LAST_RESULT
    bf16 = ml_dtypes.bfloat16
    B = query.shape[0]
    assert B == 8

    if _NC is None:
        _NC = _build(debug=bool(os.environ.get("KERNEL_DEBUG")))
        _NC.finalize()

    # weights pre-arranged to the SBUF tile layout [p, h_tile, n]
    wqk = np.concatenate([np.asarray(Wq), np.asarray(Wk)], axis=1)  # [H,128]
    wqk = np.ascontiguousarray(
        wqk.reshape(HT, P, P).transpose(1, 0, 2).reshape(P, HT * P).astype(bf16)
    )
    wv = np.ascontiguousarray(
        np.asarray(Wv)
        .reshape(HT, P, D)
        .transpose(1, 0, 2)
        .reshape(P, HT * D)
        .astype(bf16)
    )
    bq_h = np.asarray(bq, np.float32).reshape(D, 1)
    bk_h = np.asarray(bk, np.float32).reshape(D, 1)
    bv_h = np.asarray(bv, np.float32).reshape(D, 1)

    in_maps = []
    for b in range(B):
        mb = ((np.asarray(mask[b], np.float32) - 1.0) * 1e9).reshape(NT, P).T
        in_maps.append(
            {
                "qT": np.ascontiguousarray(np.asarray(query[b]).T.astype(bf16)),
                "kT": np.ascontiguousarray(np.asarray(key[b]).T.astype(bf16)),
                "vT": np.ascontiguousarray(np.asarray(value[b]).T.astype(bf16)),
                "wqk": wqk,
                "wv": wv,
                "bq": bq_h,
                "bk": bk_h,
                "bv": bv_h,
                "mb": np.ascontiguousarray(mb),
            }
        )

    res = run_bass_kernel_spmd(
        _NC,
        in_maps,
        core_ids=list(range(8)),
        trace=bool(os.environ.get("KERNEL_TRACE")),
    )
    LAST_RESULT = res
    out = np.empty((B, S, D), np.float32)
    for b in range(B):
        oT = np.asarray(res.results[b]["o"])  # [65, S] f32, unnormalized
        out[b] = (oT[:D] / oT[D : D + 1]).T
    return out
